# revision 7
# baseline (speedup 1.0000x reference)
"""Bahdanau attention layer on 8 Trainium2 NeuronCores, data-parallel over batch.

Reference computation (B=64, S=512, H=1024):
    cat    = concat([hidden_bcast, encoder_outputs], -1)        # [B,S,2H]
    energy = tanh(cat @ W.T + b)                                # [B,S,H]
    scores = energy @ v                                         # [B,S]
    attn   = softmax(scores, axis=-1)                           # [B,1,S]
    ctx    = attn @ encoder_outputs                             # [B,1,H]

Sharding: batch is split 8 ways (8 batches per core); W/b/v replicated.
Per core the dominant work is the energy matmul, computed as
P.T[h,s] = sum_k W2T[k,h] * encT[k,s] with fp32r (FP22) matmuls so the
PE streams one column per cycle.  hidden@W1.T+b collapses to a per-(h,b)
bias folded into the tanh activation.  scores are v.T @ tanh-tiles on the
PE; softmax runs on DVE/ACT; context is a fused multiply-reduce on DVE
against a PE-broadcast of the attention row.
"""

import os
import sys

if "/opt/trn_rl_repo" not in sys.path:
    sys.path.insert(0, "/opt/trn_rl_repo")

import numpy as np

B, S, H = 64, 512, 1024
N_CORES = 8
BPC = B // N_CORES
P = 128
KT = H // P  # k tiles (contraction)
HT = H // P  # h tiles (output hidden)

LAST_EXEC_TIME_NS = None
LAST_RESULTS = None

_COMPILED = {}


def _install_tile_patch():
    """This image's walrus rejects instructions with 3+ semaphore waits; Tile's
    exit drain collects one wait per active proc.  Split them across a chain of
    single-wait drains."""
    import concourse.tile as tile
    from concourse.vector_clock import ScopedClock

    if getattr(tile.TileContext, "_drain_patch_installed", False):
        return

    def _patched_drain_and_barrier(self, tick_clock, wait_clock):
        nc = self.nc
        vc = tick_clock.global_clock
        for proc in range(len(vc)):
            tick = vc[proc]
            if tick <= 0:
                continue
            d = nc.sync.drain()
            sc = ScopedClock()
            sc.require_at_least(None, proc, tick)
            wait_clock.add_sem_waits(d.ins, sc)
        nc.sync.drain()
        nc.all_engine_barrier()
        assert self.sems is not None
        popped = nc._tile_sem_poison_stack.pop()
        assert popped is self._sem_poison
        nc.clear_and_free_semaphores(list(self.sems.allocated().values()))
        nc.all_engine_barrier()

    tile.TileContext._drain_and_barrier = _patched_drain_and_barrier
    tile.TileContext._drain_patch_installed = True


def _split_excess_waits(nc, limit=1):
    """This image's walrus rejects instructions carrying more than ~2 semaphore
    waits ("Too many sync wait commands").  Move excess waits onto InstNoOp
    carriers inserted immediately before the instruction on the same engine —
    per-engine program order makes the carrier's waits complete first."""
    from concourse import mybir

    n_carriers = 0
    for f in nc.m.functions:
        for bb in f.blocks:
            insts = bb.instructions
            idx = 0
            while idx < len(insts):
                inst = insts[idx]
                si = inst.sync_info
                if si is None or len(si.on_wait) <= limit:
                    idx += 1
                    continue
                waits = list(si.on_wait)
                si.on_wait = waits[-limit:]
                extra = waits[:-limit]
                pos = idx
                for lo in range(0, len(extra), limit):
                    n_carriers += 1
                    nop = mybir.InstNoOp(
                        name=f"I-waitcarrier-{n_carriers}",
                        engine=inst.engine,
                        ins=[],
                        outs=[],
                    )
                    nop.sync_info = mybir.SyncInfo(
                        on_wait=extra[lo : lo + limit], on_update=[]
                    )
                    insts.insert(pos, nop)
                    pos += 1
                    idx += 1
                idx += 1
    return n_carriers


def _install_profile_shim():
    """antenv.axon_hooks is absent from this image; recreate it and register the
    ctypes NTFF hook so run_bass_kernel_spmd(trace=True) can profile."""
    import types

    if "antenv.axon_hooks" in sys.modules:
        return
    mod = types.ModuleType("antenv.axon_hooks")
    mod._hook = None
    mod.set_axon_ntff_profile_hook = lambda h: setattr(mod, "_hook", h)
    mod.get_axon_ntff_profile_hook = lambda: mod._hook
    sys.modules["antenv.axon_hooks"] = mod
    try:
        from trn_agent_boot.trn_boot import _ntff_profile_via_ctypes

        mod._hook = _ntff_profile_via_ctypes("/opt/axon/libaxon_pjrt.so")
    except Exception:
        pass


def _build_nc():
    import concourse.bass as bass
    import concourse.tile as tile
    from concourse import mybir
    from concourse.bass import ts
    from contextlib import ExitStack

    f32 = mybir.dt.float32
    f32r = mybir.dt.float32r
    Alu = mybir.AluOpType
    Act = mybir.ActivationFunctionType
    Ax = mybir.AxisListType

    nc = bass.Bass()
    encT = nc.declare_dram_parameter("encT", [BPC, H, S], f32, isOutput=False)
    w2t = nc.declare_dram_parameter("w2t", [H, H], f32, isOutput=False)
    w1t = nc.declare_dram_parameter("w1t", [H, H], f32, isOutput=False)
    hT = nc.declare_dram_parameter("hT", [H, BPC], f32, isOutput=False)
    bcol = nc.declare_dram_parameter("bcol", [H, 1], f32, isOutput=False)
    vcol = nc.declare_dram_parameter("vcol", [H, 1], f32, isOutput=False)
    onesrow = nc.declare_dram_parameter("onesrow", [1, P], f32, isOutput=False)
    out = nc.declare_dram_parameter("out", [BPC, H + S], f32, isOutput=True)

    with tile.TileContext(nc) as tc, ExitStack() as ctx:
        singles = ctx.enter_context(tc.tile_pool(name="singles", bufs=1))
        w2t_sb = singles.tile([P, KT, H], f32r)
        for jk in range(KT):
            nc.sync.dma_start(out=w2t_sb[:, jk, :], in_=w2t[ts(jk, P), :].bitcast(f32r))
        hT_sb = singles.tile([P, KT, BPC], f32r)
        bT_sb = singles.tile([P, HT], f32)
        vT_sb = singles.tile([P, HT], f32r)
        for jk in range(KT):
            nc.sync.dma_start(out=hT_sb[:, jk, :], in_=hT[ts(jk, P), :].bitcast(f32r))
            nc.sync.dma_start(out=bT_sb[:, jk : jk + 1], in_=bcol[ts(jk, P), :])
            nc.sync.dma_start(out=vT_sb[:, jk : jk + 1], in_=vcol[ts(jk, P), :].bitcast(f32r))
        ones_sb = singles.tile([1, P], f32r)
        nc.sync.dma_start(out=ones_sb, in_=onesrow[:, :].bitcast(f32r))
        # per-(h, b) tanh bias: hqbT[p, jh, b] = (hidden @ W1.T + b)[b, jh*128+p]
        hqbT = singles.tile([P, HT, BPC], f32)

        with tc.tile_pool(name="w1pool", bufs=3) as w1pool, tc.tile_pool(
            name="hqps", bufs=2, space="PSUM"
        ) as hqps:
            for jh in range(HT):
                ps = hqps.tile([P, BPC], f32)
                for jk in range(KT):
                    w1tile = w1pool.tile([P, P], f32r)
                    nc.sync.dma_start(out=w1tile, in_=w1t[ts(jk, P), ts(jh, P)].bitcast(f32r))
                    nc.tensor.matmul(
                        ps,
                        lhsT=w1tile,
                        rhs=hT_sb[:, jk, :],
                        start=(jk == 0),
                        stop=(jk == KT - 1),
                    )
                nc.vector.tensor_scalar(
                    out=hqbT[:, jh, :],
                    in0=ps,
                    scalar1=bT_sb[:, jh : jh + 1],
                    scalar2=None,
                    op0=Alu.add,
                )

        enc_pool = ctx.enter_context(tc.tile_pool(name="enc", bufs=3))
        et_pool = ctx.enter_context(tc.tile_pool(name="et", bufs=2))
        eps_pool = ctx.enter_context(tc.tile_pool(name="eps", bufs=2, space="PSUM"))
        scps_pool = ctx.enter_context(tc.tile_pool(name="scps", bufs=2, space="PSUM"))
        bcps_pool = ctx.enter_context(tc.tile_pool(name="bcps", bufs=2, space="PSUM"))
        sm_pool = ctx.enter_context(tc.tile_pool(name="sm", bufs=2))
        scr_pool = ctx.enter_context(tc.tile_pool(name="scr", bufs=2))
        ctxo_pool = ctx.enter_context(tc.tile_pool(name="ctxo", bufs=2))

        def emit_epilogue(b, enc_sb, sc_ps):
            # softmax over the 512 scores (single partition)
            nm = sm_pool.tile([1, 1], f32, tag="nm")
            nc.vector.reduce_max(out=nm, in_=sc_ps, axis=Ax.X, negate=True)
            a_sb = sm_pool.tile([1, S], f32r, tag="a")
            ssum = sm_pool.tile([1, 1], f32, tag="ssum")
            nc.scalar.activation(
                out=a_sb, in_=sc_ps, func=Act.Exp, bias=nm, scale=1.0, accum_out=ssum
            )
            rs = sm_pool.tile([1, 1], f32, tag="rs")
            nc.vector.reciprocal(out=rs, in_=ssum)
            nc.vector.tensor_scalar_mul(out=a_sb, in0=a_sb, scalar1=rs)
            nc.sync.dma_start(out=out[b : b + 1, H : H + S], in_=a_sb.bitcast(f32))
            # broadcast attention row to 128 partitions via K=1 matmul
            bc_ps = bcps_pool.tile([P, S], f32)
            nc.tensor.matmul(
                bc_ps,
                lhsT=ones_sb,
                rhs=a_sb,
                start=True,
                stop=True,
            )
            # context: ctxT[c] = sum_s encT[c, s] * attn[s], fused mul+reduce
            ctx_sb = ctxo_pool.tile([P, HT], f32)
            for jh in range(HT):
                scr = scr_pool.tile([P, S], f32)
                nc.vector.tensor_mul(
                    out=scr, in0=enc_sb[:, jh, :].bitcast(f32), in1=bc_ps
                )
                nc.vector.reduce_sum(
                    out=ctx_sb[:, jh : jh + 1], in_=scr, axis=Ax.X
                )
            # ctx written p-major: out[b, p*HT + j] = ctxT[p, j]; host unpermutes
            nc.sync.dma_start(
                out=out[b, 0:H].rearrange("(p j) -> p j", j=HT), in_=ctx_sb
            )

        pend = None
        for b in range(BPC):
            enc_sb = enc_pool.tile([P, KT, S], f32r)
            for jk in range(KT):
                nc.sync.dma_start(out=enc_sb[:, jk, :], in_=encT[b, ts(jk, P), :].bitcast(f32r))
            et_sb = et_pool.tile([P, HT, S], f32r)
            sc_ps = scps_pool.tile([1, S], f32)
            for jh in range(HT):
                ps = eps_pool.tile([P, S], f32)
                for jk in range(KT):
                    nc.tensor.matmul(
                        ps,
                        lhsT=w2t_sb[:, jk, ts(jh, P)],
                        rhs=enc_sb[:, jk, :],
                        start=(jk == 0),
                        stop=(jk == KT - 1),
                    )
                nc.scalar.activation(
                    out=et_sb[:, jh, :],
                    in_=ps,
                    func=Act.Tanh,
                    bias=hqbT[:, jh, b : b + 1],
                    scale=1.0,
                )
            for jh in range(HT):
                nc.tensor.matmul(
                    sc_ps,
                    lhsT=vT_sb[:, jh : jh + 1],
                    rhs=et_sb[:, jh, :],
                    start=(jh == 0),
                    stop=(jh == HT - 1),
                )
            # epilogue of the previous batch lands here so its PE work (the
            # broadcast matmul) never stalls on the softmax round-trip
            if pend is not None:
                emit_epilogue(*pend)
            pend = (b, enc_sb, sc_ps)
        emit_epilogue(*pend)

    return nc


def _get_nc():
    if "nc" not in _COMPILED:
        _install_tile_patch()
        _install_profile_shim()
        nc = _build_nc()
        _split_excess_waits(nc)
        _COMPILED["nc"] = nc
    return _COMPILED["nc"]


def kernel(hidden, encoder_outputs, W, b, v):
    global LAST_EXEC_TIME_NS, LAST_RESULTS
    from concourse.bass_utils import run_bass_kernel_spmd

    hidden = np.ascontiguousarray(np.asarray(hidden, dtype=np.float32))
    enc = np.ascontiguousarray(np.asarray(encoder_outputs, dtype=np.float32))
    W = np.ascontiguousarray(np.asarray(W, dtype=np.float32))
    b = np.ascontiguousarray(np.asarray(b, dtype=np.float32))
    v = np.ascontiguousarray(np.asarray(v, dtype=np.float32))

    w1t = np.ascontiguousarray(W[:, :H].T)  # [k, h]
    w2t = np.ascontiguousarray(W[:, H:].T)  # [k, h]
    bcol = np.ascontiguousarray(b.reshape(H, 1))
    vcol = np.ascontiguousarray(v.reshape(H, 1))

    in_maps = []
    for c in range(N_CORES):
        lo, hi = c * BPC, (c + 1) * BPC
        in_maps.append(
            {
                "encT": np.ascontiguousarray(enc[lo:hi].transpose(0, 2, 1)),
                "w2t": w2t,
                "w1t": w1t,
                "hT": np.ascontiguousarray(hidden[lo:hi].T),
                "bcol": bcol,
                "vcol": vcol,
                "onesrow": np.ones((1, P), dtype=np.float32),
            }
        )

    nc = _get_nc()
    trace = bool(os.environ.get("KERNEL_TRACE"))
    res = run_bass_kernel_spmd(
        nc,
        in_maps,
        list(range(N_CORES)),
        trace=trace,
        tmpdir=os.environ.get("KERNEL_TRACE_DIR") or None,
    )
    LAST_EXEC_TIME_NS = res.exec_time_ns
    LAST_RESULTS = res

    context = np.empty((B, 1, H), dtype=np.float32)
    attn = np.empty((B, 1, S), dtype=np.float32)
    for c in range(N_CORES):
        o = res.results[c]["out"]  # [BPC, H+S]
        lo = c * BPC
        # ctx section is stored [p, j] p-major with h = j*128 + p
        context[lo : lo + BPC, 0, :] = (
            o[:, :H].reshape(BPC, P, HT).transpose(0, 2, 1).reshape(BPC, H)
        )
        attn[lo : lo + BPC, 0, :] = o[:, H:]
    return context, attn


# revision 9
# speedup vs baseline: 1.1489x; 1.1489x over previous
"""Bahdanau attention layer on 8 Trainium2 NeuronCores, data-parallel over batch.

Reference computation (B=64, S=512, H=1024):
    cat    = concat([hidden_bcast, encoder_outputs], -1)        # [B,S,2H]
    energy = tanh(cat @ W.T + b)                                # [B,S,H]
    scores = energy @ v                                         # [B,S]
    attn   = softmax(scores, axis=-1)                           # [B,1,S]
    ctx    = attn @ encoder_outputs                             # [B,1,H]

Sharding: batch is split 8 ways (8 batches per core); W/b/v replicated.
Per core the dominant work is the energy matmul, computed as
P.T[h,s] = sum_k W2T[k,h] * encT[k,s] with fp32r (FP22) matmuls so the
PE streams one column per cycle.  hidden@W1.T+b collapses to a per-(h,b)
bias folded into the tanh activation.  scores are v.T @ tanh-tiles on the
PE; softmax runs on DVE/ACT; context is a fused multiply-reduce on DVE
against a PE-broadcast of the attention row.
"""

import os
import sys

if "/opt/trn_rl_repo" not in sys.path:
    sys.path.insert(0, "/opt/trn_rl_repo")

import numpy as np

B, S, H = 64, 512, 1024
N_CORES = 8
BPC = B // N_CORES
P = 128
KT = H // P  # k tiles (contraction)
HT = H // P  # h tiles (output hidden)

LAST_EXEC_TIME_NS = None
LAST_RESULTS = None

_COMPILED = {}


def _install_tile_patch():
    """This image's walrus rejects instructions with 3+ semaphore waits; Tile's
    exit drain collects one wait per active proc.  Split them across a chain of
    single-wait drains."""
    import concourse.tile as tile
    from concourse.vector_clock import ScopedClock

    if getattr(tile.TileContext, "_drain_patch_installed", False):
        return

    def _patched_drain_and_barrier(self, tick_clock, wait_clock):
        nc = self.nc
        vc = tick_clock.global_clock
        for proc in range(len(vc)):
            tick = vc[proc]
            if tick <= 0:
                continue
            d = nc.sync.drain()
            sc = ScopedClock()
            sc.require_at_least(None, proc, tick)
            wait_clock.add_sem_waits(d.ins, sc)
        nc.sync.drain()
        nc.all_engine_barrier()
        assert self.sems is not None
        popped = nc._tile_sem_poison_stack.pop()
        assert popped is self._sem_poison
        nc.clear_and_free_semaphores(list(self.sems.allocated().values()))
        nc.all_engine_barrier()

    tile.TileContext._drain_and_barrier = _patched_drain_and_barrier
    tile.TileContext._drain_patch_installed = True


def _split_excess_waits(nc, limit=1):
    """This image's walrus rejects instructions carrying more than ~2 semaphore
    waits ("Too many sync wait commands").  Move excess waits onto InstNoOp
    carriers inserted immediately before the instruction on the same engine —
    per-engine program order makes the carrier's waits complete first."""
    from concourse import mybir

    n_carriers = 0
    for f in nc.m.functions:
        for bb in f.blocks:
            insts = bb.instructions
            idx = 0
            while idx < len(insts):
                inst = insts[idx]
                si = inst.sync_info
                if si is None or len(si.on_wait) <= limit:
                    idx += 1
                    continue
                waits = list(si.on_wait)
                si.on_wait = waits[-limit:]
                extra = waits[:-limit]
                pos = idx
                for lo in range(0, len(extra), limit):
                    n_carriers += 1
                    nop = mybir.InstNoOp(
                        name=f"I-waitcarrier-{n_carriers}",
                        engine=inst.engine,
                        ins=[],
                        outs=[],
                    )
                    nop.sync_info = mybir.SyncInfo(
                        on_wait=extra[lo : lo + limit], on_update=[]
                    )
                    insts.insert(pos, nop)
                    pos += 1
                    idx += 1
                idx += 1
    return n_carriers


def _install_profile_shim():
    """antenv.axon_hooks is absent from this image; recreate it and register the
    ctypes NTFF hook so run_bass_kernel_spmd(trace=True) can profile."""
    import types

    if "antenv.axon_hooks" in sys.modules:
        return
    mod = types.ModuleType("antenv.axon_hooks")
    mod._hook = None
    mod.set_axon_ntff_profile_hook = lambda h: setattr(mod, "_hook", h)
    mod.get_axon_ntff_profile_hook = lambda: mod._hook
    sys.modules["antenv.axon_hooks"] = mod
    try:
        from trn_agent_boot.trn_boot import _ntff_profile_via_ctypes

        mod._hook = _ntff_profile_via_ctypes("/opt/axon/libaxon_pjrt.so")
    except Exception:
        pass


def _build_nc():
    import concourse.bass as bass
    import concourse.tile as tile
    from concourse import mybir
    from concourse.bass import ts
    from contextlib import ExitStack

    f32 = mybir.dt.float32
    f32r = mybir.dt.float32r
    Alu = mybir.AluOpType
    Act = mybir.ActivationFunctionType
    Ax = mybir.AxisListType

    nc = bass.Bass()
    encT = nc.declare_dram_parameter("encT", [BPC, H, S], f32, isOutput=False)
    w2t = nc.declare_dram_parameter("w2t", [H, H], f32, isOutput=False)
    w1t = nc.declare_dram_parameter("w1t", [H, H], f32, isOutput=False)
    hT = nc.declare_dram_parameter("hT", [H, BPC], f32, isOutput=False)
    bcol = nc.declare_dram_parameter("bcol", [H, 1], f32, isOutput=False)
    vcol = nc.declare_dram_parameter("vcol", [H, 1], f32, isOutput=False)
    onesrow = nc.declare_dram_parameter("onesrow", [1, P], f32, isOutput=False)
    out = nc.declare_dram_parameter("out", [BPC, H + S], f32, isOutput=True)

    with tile.TileContext(nc) as tc, ExitStack() as ctx:
        singles = ctx.enter_context(tc.tile_pool(name="singles", bufs=1))
        # W1 + the small tensors first in DMA order: the hq prologue can start
        # on the PE ~2us in, while W2T/enc stream behind them.
        hT_sb = singles.tile([P, KT, BPC], f32r)
        bT_sb = singles.tile([P, HT], f32)
        vT_sb = singles.tile([P, HT], f32r)
        for jk in range(KT):
            nc.sync.dma_start(out=hT_sb[:, jk, :], in_=hT[ts(jk, P), :].bitcast(f32r))
            nc.sync.dma_start(out=bT_sb[:, jk : jk + 1], in_=bcol[ts(jk, P), :])
            nc.sync.dma_start(out=vT_sb[:, jk : jk + 1], in_=vcol[ts(jk, P), :].bitcast(f32r))
        ones_sb = singles.tile([1, P], f32r)
        nc.sync.dma_start(out=ones_sb, in_=onesrow[:, :].bitcast(f32r))
        # per-(h, b) tanh bias: hqbT[p, jh, b] = (hidden @ W1.T + b)[b, jh*128+p]
        hqbT = singles.tile([P, HT, BPC], f32)
        w2t_sb = singles.tile([P, KT, H], f32r)

        with tc.tile_pool(name="w1pool", bufs=1) as w1pool, tc.tile_pool(
            name="hqps", bufs=2, space="PSUM"
        ) as hqps:
            w1_sb = w1pool.tile([P, KT, H], f32r)
            for jk in range(KT):
                nc.sync.dma_start(
                    out=w1_sb[:, jk, :], in_=w1t[ts(jk, P), :].bitcast(f32r)
                )
            for jh in range(HT):
                ps = hqps.tile([P, BPC], f32)
                for jk in range(KT):
                    nc.tensor.matmul(
                        ps,
                        lhsT=w1_sb[:, jk, ts(jh, P)],
                        rhs=hT_sb[:, jk, :],
                        start=(jk == 0),
                        stop=(jk == KT - 1),
                    )
                nc.vector.tensor_scalar(
                    out=hqbT[:, jh, :],
                    in0=ps,
                    scalar1=bT_sb[:, jh : jh + 1],
                    scalar2=None,
                    op0=Alu.add,
                )

        enc_pool = ctx.enter_context(tc.tile_pool(name="enc", bufs=3))
        et_pool = ctx.enter_context(tc.tile_pool(name="et", bufs=2))
        eps_pool = ctx.enter_context(tc.tile_pool(name="eps", bufs=2, space="PSUM"))
        scps_pool = ctx.enter_context(tc.tile_pool(name="scps", bufs=2, space="PSUM"))
        bcps_pool = ctx.enter_context(tc.tile_pool(name="bcps", bufs=2, space="PSUM"))
        sm_pool = ctx.enter_context(tc.tile_pool(name="sm", bufs=2))
        scr_pool = ctx.enter_context(tc.tile_pool(name="scr", bufs=2))
        ctxo_pool = ctx.enter_context(tc.tile_pool(name="ctxo", bufs=2))

        def emit_epilogue(b, enc_sb, sc_ps):
            # softmax over the 512 scores (single partition)
            nm = sm_pool.tile([1, 1], f32, tag="nm")
            nc.vector.reduce_max(out=nm, in_=sc_ps, axis=Ax.X, negate=True)
            a_sb = sm_pool.tile([1, S], f32r, tag="a")
            ssum = sm_pool.tile([1, 1], f32, tag="ssum")
            nc.scalar.activation(
                out=a_sb, in_=sc_ps, func=Act.Exp, bias=nm, scale=1.0, accum_out=ssum
            )
            rs = sm_pool.tile([1, 1], f32, tag="rs")
            nc.vector.reciprocal(out=rs, in_=ssum)
            nc.vector.tensor_scalar_mul(out=a_sb, in0=a_sb, scalar1=rs)
            nc.sync.dma_start(out=out[b : b + 1, H : H + S], in_=a_sb.bitcast(f32))
            # broadcast attention row to 128 partitions via K=1 matmul
            bc_ps = bcps_pool.tile([P, S], f32)
            nc.tensor.matmul(
                bc_ps,
                lhsT=ones_sb,
                rhs=a_sb,
                start=True,
                stop=True,
            )
            # context: ctxT[c] = sum_s encT[c, s] * attn[s], fused mul+reduce
            ctx_sb = ctxo_pool.tile([P, HT], f32)
            for jh in range(HT):
                scr = scr_pool.tile([P, S], f32)
                nc.vector.tensor_mul(
                    out=scr, in0=enc_sb[:, jh, :].bitcast(f32), in1=bc_ps
                )
                nc.vector.reduce_sum(
                    out=ctx_sb[:, jh : jh + 1], in_=scr, axis=Ax.X
                )
            # ctx written p-major: out[b, p*HT + j] = ctxT[p, j]; host unpermutes
            nc.sync.dma_start(
                out=out[b, 0:H].rearrange("(p j) -> p j", j=HT), in_=ctx_sb
            )

        pend = None
        for b in range(BPC):
            enc_sb = enc_pool.tile([P, KT, S], f32r)
            for jk in range(KT):
                nc.sync.dma_start(out=enc_sb[:, jk, :], in_=encT[b, ts(jk, P), :].bitcast(f32r))
                if b == 0:
                    # stream W2T chunk-by-chunk alongside batch 0's encoder
                    # tiles so the first energy matmuls start ~3us in
                    nc.sync.dma_start(
                        out=w2t_sb[:, jk, :], in_=w2t[ts(jk, P), :].bitcast(f32r)
                    )
            et_sb = et_pool.tile([P, HT, S], f32r)
            sc_ps = scps_pool.tile([1, S], f32)
            for jh in range(HT):
                ps = eps_pool.tile([P, S], f32)
                for jk in range(KT):
                    nc.tensor.matmul(
                        ps,
                        lhsT=w2t_sb[:, jk, ts(jh, P)],
                        rhs=enc_sb[:, jk, :],
                        start=(jk == 0),
                        stop=(jk == KT - 1),
                    )
                nc.scalar.activation(
                    out=et_sb[:, jh, :],
                    in_=ps,
                    func=Act.Tanh,
                    bias=hqbT[:, jh, b : b + 1],
                    scale=1.0,
                )
            for jh in range(HT):
                nc.tensor.matmul(
                    sc_ps,
                    lhsT=vT_sb[:, jh : jh + 1],
                    rhs=et_sb[:, jh, :],
                    start=(jh == 0),
                    stop=(jh == HT - 1),
                )
            # epilogue of the previous batch lands here so its PE work (the
            # broadcast matmul) never stalls on the softmax round-trip
            if pend is not None:
                emit_epilogue(*pend)
            pend = (b, enc_sb, sc_ps)
        emit_epilogue(*pend)

    return nc


def _get_nc():
    if "nc" not in _COMPILED:
        _install_tile_patch()
        _install_profile_shim()
        nc = _build_nc()
        _split_excess_waits(nc)
        _COMPILED["nc"] = nc
    return _COMPILED["nc"]


def kernel(hidden, encoder_outputs, W, b, v):
    global LAST_EXEC_TIME_NS, LAST_RESULTS
    from concourse.bass_utils import run_bass_kernel_spmd

    hidden = np.ascontiguousarray(np.asarray(hidden, dtype=np.float32))
    enc = np.ascontiguousarray(np.asarray(encoder_outputs, dtype=np.float32))
    W = np.ascontiguousarray(np.asarray(W, dtype=np.float32))
    b = np.ascontiguousarray(np.asarray(b, dtype=np.float32))
    v = np.ascontiguousarray(np.asarray(v, dtype=np.float32))

    w1t = np.ascontiguousarray(W[:, :H].T)  # [k, h]
    w2t = np.ascontiguousarray(W[:, H:].T)  # [k, h]
    bcol = np.ascontiguousarray(b.reshape(H, 1))
    vcol = np.ascontiguousarray(v.reshape(H, 1))

    in_maps = []
    for c in range(N_CORES):
        lo, hi = c * BPC, (c + 1) * BPC
        in_maps.append(
            {
                "encT": np.ascontiguousarray(enc[lo:hi].transpose(0, 2, 1)),
                "w2t": w2t,
                "w1t": w1t,
                "hT": np.ascontiguousarray(hidden[lo:hi].T),
                "bcol": bcol,
                "vcol": vcol,
                "onesrow": np.ones((1, P), dtype=np.float32),
            }
        )

    nc = _get_nc()
    trace = bool(os.environ.get("KERNEL_TRACE"))
    res = run_bass_kernel_spmd(
        nc,
        in_maps,
        list(range(N_CORES)),
        trace=trace,
        tmpdir=os.environ.get("KERNEL_TRACE_DIR") or None,
    )
    LAST_EXEC_TIME_NS = res.exec_time_ns
    LAST_RESULTS = res

    context = np.empty((B, 1, H), dtype=np.float32)
    attn = np.empty((B, 1, S), dtype=np.float32)
    for c in range(N_CORES):
        o = res.results[c]["out"]  # [BPC, H+S]
        lo = c * BPC
        # ctx section is stored [p, j] p-major with h = j*128 + p
        context[lo : lo + BPC, 0, :] = (
            o[:, :H].reshape(BPC, P, HT).transpose(0, 2, 1).reshape(BPC, H)
        )
        attn[lo : lo + BPC, 0, :] = o[:, H:]
    return context, attn


# revision 11
# speedup vs baseline: 1.1551x; 1.0054x over previous
"""Bahdanau attention layer on 8 Trainium2 NeuronCores, data-parallel over batch.

Reference computation (B=64, S=512, H=1024):
    cat    = concat([hidden_bcast, encoder_outputs], -1)        # [B,S,2H]
    energy = tanh(cat @ W.T + b)                                # [B,S,H]
    scores = energy @ v                                         # [B,S]
    attn   = softmax(scores, axis=-1)                           # [B,1,S]
    ctx    = attn @ encoder_outputs                             # [B,1,H]

Sharding: batch is split 8 ways (8 batches per core); W/b/v replicated.
Per core the dominant work is the energy matmul, computed as
P.T[h,s] = sum_k W2T[k,h] * encT[k,s] with fp32r (FP22) matmuls so the
PE streams one column per cycle.  hidden@W1.T+b collapses to a per-(h,b)
bias folded into the tanh activation.  scores are v.T @ tanh-tiles on the
PE; softmax runs on DVE/ACT; context is a fused multiply-reduce on DVE
against a PE-broadcast of the attention row.
"""

import os
import sys

if "/opt/trn_rl_repo" not in sys.path:
    sys.path.insert(0, "/opt/trn_rl_repo")

import numpy as np

B, S, H = 64, 512, 1024
N_CORES = 8
BPC = B // N_CORES
P = 128
KT = H // P  # k tiles (contraction)
HT = H // P  # h tiles (output hidden)

LAST_EXEC_TIME_NS = None
LAST_RESULTS = None

_COMPILED = {}


def _install_tile_patch():
    """This image's walrus rejects instructions with 3+ semaphore waits; Tile's
    exit drain collects one wait per active proc.  Split them across a chain of
    single-wait drains."""
    import concourse.tile as tile
    from concourse.vector_clock import ScopedClock

    if getattr(tile.TileContext, "_drain_patch_installed", False):
        return

    def _patched_drain_and_barrier(self, tick_clock, wait_clock):
        nc = self.nc
        vc = tick_clock.global_clock
        for proc in range(len(vc)):
            tick = vc[proc]
            if tick <= 0:
                continue
            d = nc.sync.drain()
            sc = ScopedClock()
            sc.require_at_least(None, proc, tick)
            wait_clock.add_sem_waits(d.ins, sc)
        nc.sync.drain()
        nc.all_engine_barrier()
        assert self.sems is not None
        popped = nc._tile_sem_poison_stack.pop()
        assert popped is self._sem_poison
        nc.clear_and_free_semaphores(list(self.sems.allocated().values()))
        nc.all_engine_barrier()

    tile.TileContext._drain_and_barrier = _patched_drain_and_barrier
    tile.TileContext._drain_patch_installed = True


def _split_excess_waits(nc, limit=1):
    """This image's walrus rejects instructions carrying more than ~2 semaphore
    waits ("Too many sync wait commands").  Move excess waits onto InstNoOp
    carriers inserted immediately before the instruction on the same engine —
    per-engine program order makes the carrier's waits complete first."""
    from concourse import mybir

    n_carriers = 0
    for f in nc.m.functions:
        for bb in f.blocks:
            insts = bb.instructions
            idx = 0
            while idx < len(insts):
                inst = insts[idx]
                si = inst.sync_info
                if si is None or len(si.on_wait) <= limit:
                    idx += 1
                    continue
                waits = list(si.on_wait)
                si.on_wait = waits[-limit:]
                extra = waits[:-limit]
                pos = idx
                for lo in range(0, len(extra), limit):
                    n_carriers += 1
                    nop = mybir.InstNoOp(
                        name=f"I-waitcarrier-{n_carriers}",
                        engine=inst.engine,
                        ins=[],
                        outs=[],
                    )
                    nop.sync_info = mybir.SyncInfo(
                        on_wait=extra[lo : lo + limit], on_update=[]
                    )
                    insts.insert(pos, nop)
                    pos += 1
                    idx += 1
                idx += 1
    return n_carriers


def _install_profile_shim():
    """antenv.axon_hooks is absent from this image; recreate it and register the
    ctypes NTFF hook so run_bass_kernel_spmd(trace=True) can profile."""
    import types

    if "antenv.axon_hooks" in sys.modules:
        return
    mod = types.ModuleType("antenv.axon_hooks")
    mod._hook = None
    mod.set_axon_ntff_profile_hook = lambda h: setattr(mod, "_hook", h)
    mod.get_axon_ntff_profile_hook = lambda: mod._hook
    sys.modules["antenv.axon_hooks"] = mod
    try:
        from trn_agent_boot.trn_boot import _ntff_profile_via_ctypes

        mod._hook = _ntff_profile_via_ctypes("/opt/axon/libaxon_pjrt.so")
    except Exception:
        pass


def _build_nc():
    import concourse.bass as bass
    import concourse.tile as tile
    from concourse import mybir
    from concourse.bass import ts
    from contextlib import ExitStack

    f32 = mybir.dt.float32
    f32r = mybir.dt.float32r
    Alu = mybir.AluOpType
    Act = mybir.ActivationFunctionType
    Ax = mybir.AxisListType

    nc = bass.Bass()
    encT = nc.declare_dram_parameter("encT", [BPC, H, S], f32, isOutput=False)
    w2t = nc.declare_dram_parameter("w2t", [H, H], f32, isOutput=False)
    w1t = nc.declare_dram_parameter("w1t", [H, H], f32, isOutput=False)
    smalls = nc.declare_dram_parameter("smalls", [H, BPC + 2], f32, isOutput=False)
    onesrow = nc.declare_dram_parameter("onesrow", [1, P], f32, isOutput=False)
    out = nc.declare_dram_parameter("out", [BPC, H + S], f32, isOutput=True)

    with tile.TileContext(nc) as tc, ExitStack() as ctx:
        singles = ctx.enter_context(tc.tile_pool(name="singles", bufs=1))
        # One DMA for all the small tensors (hT cols 0..7, b col 8, v col 9):
        # tiny transfers each cost ~600ns of SP issue, so batch them.
        sm_sb = singles.tile([P, KT, BPC + 2], f32r)
        nc.sync.dma_start(
            out=sm_sb,
            in_=smalls.rearrange("(j p) c -> p j c", p=P).bitcast(f32r),
        )
        ones_sb = singles.tile([1, P], f32r)
        nc.sync.dma_start(out=ones_sb, in_=onesrow[:, :].bitcast(f32r))
        # per-(h, b) tanh bias: hqbT[p, jh, b] = (hidden @ W1.T + b)[b, jh*128+p]
        hqbT = singles.tile([P, HT, BPC], f32)
        w2t_sb = singles.tile([P, KT, H], f32r)

        with tc.tile_pool(name="w1pool", bufs=1) as w1pool, tc.tile_pool(
            name="hqps", bufs=2, space="PSUM"
        ) as hqps:
            w1_sb = w1pool.tile([P, KT, H], f32r)
            for jk in range(KT):
                nc.sync.dma_start(
                    out=w1_sb[:, jk, :], in_=w1t[ts(jk, P), :].bitcast(f32r)
                )
            for jh in range(HT):
                ps = hqps.tile([P, BPC], f32)
                for jk in range(KT):
                    nc.tensor.matmul(
                        ps,
                        lhsT=w1_sb[:, jk, ts(jh, P)],
                        rhs=sm_sb[:, jk, 0:BPC],
                        start=(jk == 0),
                        stop=(jk == KT - 1),
                    )
                nc.vector.tensor_scalar(
                    out=hqbT[:, jh, :],
                    in0=ps,
                    scalar1=sm_sb[:, jh, BPC : BPC + 1].bitcast(f32),
                    scalar2=None,
                    op0=Alu.add,
                )

        enc_pool = ctx.enter_context(tc.tile_pool(name="enc", bufs=3))
        et_pool = ctx.enter_context(tc.tile_pool(name="et", bufs=2))
        eps_pool = ctx.enter_context(tc.tile_pool(name="eps", bufs=2, space="PSUM"))
        scps_pool = ctx.enter_context(tc.tile_pool(name="scps", bufs=2, space="PSUM"))
        bcps_pool = ctx.enter_context(tc.tile_pool(name="bcps", bufs=2, space="PSUM"))
        sm_pool = ctx.enter_context(tc.tile_pool(name="sm", bufs=2))
        scr_pool = ctx.enter_context(tc.tile_pool(name="scr", bufs=2))
        ctxo_pool = ctx.enter_context(tc.tile_pool(name="ctxo", bufs=2))

        def emit_epilogue(b, enc_sb, sc_ps):
            # softmax over the 512 scores (single partition)
            nm = sm_pool.tile([1, 1], f32, tag="nm")
            nc.vector.reduce_max(out=nm, in_=sc_ps, axis=Ax.X, negate=True)
            a_sb = sm_pool.tile([1, S], f32r, tag="a")
            ssum = sm_pool.tile([1, 1], f32, tag="ssum")
            nc.scalar.activation(
                out=a_sb, in_=sc_ps, func=Act.Exp, bias=nm, scale=1.0, accum_out=ssum
            )
            rs = sm_pool.tile([1, 1], f32, tag="rs")
            nc.vector.reciprocal(out=rs, in_=ssum)
            nc.vector.tensor_scalar_mul(out=a_sb, in0=a_sb, scalar1=rs)
            nc.sync.dma_start(out=out[b : b + 1, H : H + S], in_=a_sb.bitcast(f32))
            # broadcast attention row to 128 partitions via K=1 matmul
            bc_ps = bcps_pool.tile([P, S], f32)
            nc.tensor.matmul(
                bc_ps,
                lhsT=ones_sb,
                rhs=a_sb,
                start=True,
                stop=True,
            )
            # context: ctxT[c] = sum_s encT[c, s] * attn[s], fused mul+reduce
            ctx_sb = ctxo_pool.tile([P, HT], f32)
            for jh in range(HT):
                scr = scr_pool.tile([P, S], f32)
                nc.vector.tensor_mul(
                    out=scr, in0=enc_sb[:, jh, :].bitcast(f32), in1=bc_ps
                )
                nc.vector.reduce_sum(
                    out=ctx_sb[:, jh : jh + 1], in_=scr, axis=Ax.X
                )
            # ctx written p-major: out[b, p*HT + j] = ctxT[p, j]; host unpermutes
            nc.sync.dma_start(
                out=out[b, 0:H].rearrange("(p j) -> p j", j=HT), in_=ctx_sb
            )

        pend = None
        for b in range(BPC):
            enc_sb = enc_pool.tile([P, KT, S], f32r)
            if b == 0:
                # chunked + interleaved with W2T so the first energy matmuls
                # start as soon as the first chunk pair lands
                for jk in range(KT):
                    nc.sync.dma_start(
                        out=enc_sb[:, jk, :], in_=encT[b, ts(jk, P), :].bitcast(f32r)
                    )
                    nc.sync.dma_start(
                        out=w2t_sb[:, jk, :], in_=w2t[ts(jk, P), :].bitcast(f32r)
                    )
            else:
                nc.sync.dma_start(
                    out=enc_sb,
                    in_=encT[b].rearrange("(j p) s -> p j s", p=P).bitcast(f32r),
                )
            et_sb = et_pool.tile([P, HT, S], f32r)
            sc_ps = scps_pool.tile([1, S], f32)
            for jh in range(HT):
                ps = eps_pool.tile([P, S], f32)
                for jk in range(KT):
                    nc.tensor.matmul(
                        ps,
                        lhsT=w2t_sb[:, jk, ts(jh, P)],
                        rhs=enc_sb[:, jk, :],
                        start=(jk == 0),
                        stop=(jk == KT - 1),
                    )
                nc.scalar.activation(
                    out=et_sb[:, jh, :],
                    in_=ps,
                    func=Act.Tanh,
                    bias=hqbT[:, jh, b : b + 1],
                    scale=1.0,
                )
            for jh in range(HT):
                nc.tensor.matmul(
                    sc_ps,
                    lhsT=sm_sb[:, jh, BPC + 1 : BPC + 2],
                    rhs=et_sb[:, jh, :],
                    start=(jh == 0),
                    stop=(jh == HT - 1),
                )
            # epilogue of the previous batch lands here so its PE work (the
            # broadcast matmul) never stalls on the softmax round-trip
            if pend is not None:
                emit_epilogue(*pend)
            pend = (b, enc_sb, sc_ps)
        emit_epilogue(*pend)

    return nc


def _get_nc():
    if "nc" not in _COMPILED:
        _install_tile_patch()
        _install_profile_shim()
        nc = _build_nc()
        _split_excess_waits(nc)
        _COMPILED["nc"] = nc
    return _COMPILED["nc"]


def kernel(hidden, encoder_outputs, W, b, v):
    global LAST_EXEC_TIME_NS, LAST_RESULTS
    from concourse.bass_utils import run_bass_kernel_spmd

    hidden = np.ascontiguousarray(np.asarray(hidden, dtype=np.float32))
    enc = np.ascontiguousarray(np.asarray(encoder_outputs, dtype=np.float32))
    W = np.ascontiguousarray(np.asarray(W, dtype=np.float32))
    b = np.ascontiguousarray(np.asarray(b, dtype=np.float32))
    v = np.ascontiguousarray(np.asarray(v, dtype=np.float32))

    w1t = np.ascontiguousarray(W[:, :H].T)  # [k, h]
    w2t = np.ascontiguousarray(W[:, H:].T)  # [k, h]

    in_maps = []
    for c in range(N_CORES):
        lo, hi = c * BPC, (c + 1) * BPC
        in_maps.append(
            {
                "encT": np.ascontiguousarray(enc[lo:hi].transpose(0, 2, 1)),
                "w2t": w2t,
                "w1t": w1t,
                "smalls": np.ascontiguousarray(
                    np.concatenate(
                        [hidden[lo:hi].T, b.reshape(H, 1), v.reshape(H, 1)], axis=1
                    )
                ),
                "onesrow": np.ones((1, P), dtype=np.float32),
            }
        )

    nc = _get_nc()
    trace = bool(os.environ.get("KERNEL_TRACE"))
    res = run_bass_kernel_spmd(
        nc,
        in_maps,
        list(range(N_CORES)),
        trace=trace,
        tmpdir=os.environ.get("KERNEL_TRACE_DIR") or None,
    )
    LAST_EXEC_TIME_NS = res.exec_time_ns
    LAST_RESULTS = res

    context = np.empty((B, 1, H), dtype=np.float32)
    attn = np.empty((B, 1, S), dtype=np.float32)
    for c in range(N_CORES):
        o = res.results[c]["out"]  # [BPC, H+S]
        lo = c * BPC
        # ctx section is stored [p, j] p-major with h = j*128 + p
        context[lo : lo + BPC, 0, :] = (
            o[:, :H].reshape(BPC, P, HT).transpose(0, 2, 1).reshape(BPC, H)
        )
        attn[lo : lo + BPC, 0, :] = o[:, H:]
    return context, attn


# revision 12
# speedup vs baseline: 1.1570x; 1.0016x over previous
"""Bahdanau attention layer on 8 Trainium2 NeuronCores, data-parallel over batch.

Reference computation (B=64, S=512, H=1024):
    cat    = concat([hidden_bcast, encoder_outputs], -1)        # [B,S,2H]
    energy = tanh(cat @ W.T + b)                                # [B,S,H]
    scores = energy @ v                                         # [B,S]
    attn   = softmax(scores, axis=-1)                           # [B,1,S]
    ctx    = attn @ encoder_outputs                             # [B,1,H]

Sharding: batch is split 8 ways (8 batches per core); W/b/v replicated.
Per core the dominant work is the energy matmul, computed as
P.T[h,s] = sum_k W2T[k,h] * encT[k,s] with fp32r (FP22) matmuls so the
PE streams one column per cycle.  hidden@W1.T+b collapses to a per-(h,b)
bias folded into the tanh activation.  scores are v.T @ tanh-tiles on the
PE; softmax runs on DVE/ACT; context is a fused multiply-reduce on DVE
against a PE-broadcast of the attention row.
"""

import os
import sys

if "/opt/trn_rl_repo" not in sys.path:
    sys.path.insert(0, "/opt/trn_rl_repo")

import numpy as np

B, S, H = 64, 512, 1024
N_CORES = 8
BPC = B // N_CORES
P = 128
KT = H // P  # k tiles (contraction)
HT = H // P  # h tiles (output hidden)

LAST_EXEC_TIME_NS = None
LAST_RESULTS = None

_COMPILED = {}


def _install_tile_patch():
    """This image's walrus rejects instructions with 3+ semaphore waits; Tile's
    exit drain collects one wait per active proc.  Split them across a chain of
    single-wait drains."""
    import concourse.tile as tile
    from concourse.vector_clock import ScopedClock

    if getattr(tile.TileContext, "_drain_patch_installed", False):
        return

    def _patched_drain_and_barrier(self, tick_clock, wait_clock):
        nc = self.nc
        vc = tick_clock.global_clock
        for proc in range(len(vc)):
            tick = vc[proc]
            if tick <= 0:
                continue
            d = nc.sync.drain()
            sc = ScopedClock()
            sc.require_at_least(None, proc, tick)
            wait_clock.add_sem_waits(d.ins, sc)
        nc.sync.drain()
        nc.all_engine_barrier()
        assert self.sems is not None
        popped = nc._tile_sem_poison_stack.pop()
        assert popped is self._sem_poison
        nc.clear_and_free_semaphores(list(self.sems.allocated().values()))
        nc.all_engine_barrier()

    tile.TileContext._drain_and_barrier = _patched_drain_and_barrier
    tile.TileContext._drain_patch_installed = True


def _split_excess_waits(nc, limit=1):
    """This image's walrus rejects instructions carrying more than ~2 semaphore
    waits ("Too many sync wait commands").  Move excess waits onto InstNoOp
    carriers inserted immediately before the instruction on the same engine —
    per-engine program order makes the carrier's waits complete first."""
    from concourse import mybir

    n_carriers = 0
    for f in nc.m.functions:
        for bb in f.blocks:
            insts = bb.instructions
            idx = 0
            while idx < len(insts):
                inst = insts[idx]
                si = inst.sync_info
                if si is None or len(si.on_wait) <= limit:
                    idx += 1
                    continue
                waits = list(si.on_wait)
                si.on_wait = waits[-limit:]
                extra = waits[:-limit]
                pos = idx
                for lo in range(0, len(extra), limit):
                    n_carriers += 1
                    nop = mybir.InstNoOp(
                        name=f"I-waitcarrier-{n_carriers}",
                        engine=inst.engine,
                        ins=[],
                        outs=[],
                    )
                    nop.sync_info = mybir.SyncInfo(
                        on_wait=extra[lo : lo + limit], on_update=[]
                    )
                    insts.insert(pos, nop)
                    pos += 1
                    idx += 1
                idx += 1
    return n_carriers


def _install_profile_shim():
    """antenv.axon_hooks is absent from this image; recreate it and register the
    ctypes NTFF hook so run_bass_kernel_spmd(trace=True) can profile."""
    import types

    if "antenv.axon_hooks" in sys.modules:
        return
    mod = types.ModuleType("antenv.axon_hooks")
    mod._hook = None
    mod.set_axon_ntff_profile_hook = lambda h: setattr(mod, "_hook", h)
    mod.get_axon_ntff_profile_hook = lambda: mod._hook
    sys.modules["antenv.axon_hooks"] = mod
    try:
        from trn_agent_boot.trn_boot import _ntff_profile_via_ctypes

        mod._hook = _ntff_profile_via_ctypes("/opt/axon/libaxon_pjrt.so")
    except Exception:
        pass


def _build_nc():
    import concourse.bass as bass
    import concourse.tile as tile
    from concourse import mybir
    from concourse.bass import ts
    from contextlib import ExitStack

    f32 = mybir.dt.float32
    f32r = mybir.dt.float32r
    Alu = mybir.AluOpType
    Act = mybir.ActivationFunctionType
    Ax = mybir.AxisListType

    nc = bass.Bass()
    encT = nc.declare_dram_parameter("encT", [BPC, H, S], f32, isOutput=False)
    w2t = nc.declare_dram_parameter("w2t", [H, H], f32, isOutput=False)
    w1t = nc.declare_dram_parameter("w1t", [H, H], f32, isOutput=False)
    smalls = nc.declare_dram_parameter("smalls", [H, BPC + 2], f32, isOutput=False)
    onesrow = nc.declare_dram_parameter("onesrow", [1, P], f32, isOutput=False)
    out = nc.declare_dram_parameter("out", [BPC, H + S], f32, isOutput=True)

    with tile.TileContext(nc) as tc, ExitStack() as ctx:
        singles = ctx.enter_context(tc.tile_pool(name="singles", bufs=1))
        # One DMA for all the small tensors (hT cols 0..7, b col 8, v col 9):
        # tiny transfers each cost ~600ns of SP issue, so batch them.
        sm_sb = singles.tile([P, KT, BPC + 2], f32r)
        nc.sync.dma_start(
            out=sm_sb,
            in_=smalls.rearrange("(j p) c -> p j c", p=P).bitcast(f32r),
        )
        ones_sb = singles.tile([1, P], f32r)
        nc.sync.dma_start(out=ones_sb, in_=onesrow[:, :].bitcast(f32r))
        # per-(h, b) tanh bias: hqbT[p, jh, b] = (hidden @ W1.T + b)[b, jh*128+p]
        hqbT = singles.tile([P, HT, BPC], f32)
        w2t_sb = singles.tile([P, KT, H], f32r)

        with tc.tile_pool(name="w1pool", bufs=1) as w1pool, tc.tile_pool(
            name="hqps", bufs=2, space="PSUM"
        ) as hqps:
            w1_sb = w1pool.tile([P, KT, H], f32r)
            for jk in range(KT):
                nc.sync.dma_start(
                    out=w1_sb[:, jk, :], in_=w1t[ts(jk, P), :].bitcast(f32r)
                )
            for jh in range(HT):
                ps = hqps.tile([P, BPC], f32)
                for jk in range(KT):
                    nc.tensor.matmul(
                        ps,
                        lhsT=w1_sb[:, jk, ts(jh, P)],
                        rhs=sm_sb[:, jk, 0:BPC],
                        start=(jk == 0),
                        stop=(jk == KT - 1),
                    )
                nc.vector.tensor_scalar(
                    out=hqbT[:, jh, :],
                    in0=ps,
                    scalar1=sm_sb[:, jh, BPC : BPC + 1].bitcast(f32),
                    scalar2=None,
                    op0=Alu.add,
                )

        enc_pool = ctx.enter_context(tc.tile_pool(name="enc", bufs=3))
        et_pool = ctx.enter_context(tc.tile_pool(name="et", bufs=2))
        eps_pool = ctx.enter_context(tc.tile_pool(name="eps", bufs=2, space="PSUM"))
        scps_pool = ctx.enter_context(tc.tile_pool(name="scps", bufs=2, space="PSUM"))
        bcps_pool = ctx.enter_context(tc.tile_pool(name="bcps", bufs=2, space="PSUM"))
        sm_pool = ctx.enter_context(tc.tile_pool(name="sm", bufs=2))
        scr_pool = ctx.enter_context(tc.tile_pool(name="scr", bufs=2))
        ctxo_pool = ctx.enter_context(tc.tile_pool(name="ctxo", bufs=2))

        def emit_epilogue(b, enc_sb, sc_ps):
            # softmax over the 512 scores (single partition)
            nm = sm_pool.tile([1, 1], f32, tag="nm")
            nc.vector.reduce_max(out=nm, in_=sc_ps, axis=Ax.X, negate=True)
            a_sb = sm_pool.tile([1, S], f32r, tag="a")
            ssum = sm_pool.tile([1, 1], f32, tag="ssum")
            nc.scalar.activation(
                out=a_sb, in_=sc_ps, func=Act.Exp, bias=nm, scale=1.0, accum_out=ssum
            )
            rs = sm_pool.tile([1, 1], f32, tag="rs")
            nc.vector.reciprocal(out=rs, in_=ssum)
            nc.vector.tensor_scalar_mul(out=a_sb, in0=a_sb, scalar1=rs)
            nc.gpsimd.dma_start(out=out[b : b + 1, H : H + S], in_=a_sb.bitcast(f32))
            # broadcast attention row to 128 partitions via K=1 matmul
            bc_ps = bcps_pool.tile([P, S], f32)
            nc.tensor.matmul(
                bc_ps,
                lhsT=ones_sb,
                rhs=a_sb,
                start=True,
                stop=True,
            )
            # context: ctxT[c] = sum_s encT[c, s] * attn[s], fused mul+reduce
            ctx_sb = ctxo_pool.tile([P, HT], f32)
            for jh in range(HT):
                scr = scr_pool.tile([P, S], f32)
                nc.vector.tensor_mul(
                    out=scr, in0=enc_sb[:, jh, :].bitcast(f32), in1=bc_ps
                )
                nc.vector.reduce_sum(
                    out=ctx_sb[:, jh : jh + 1], in_=scr, axis=Ax.X
                )
            # ctx written p-major: out[b, p*HT + j] = ctxT[p, j]; host unpermutes
            nc.gpsimd.dma_start(
                out=out[b, 0:H].rearrange("(p j) -> p j", j=HT), in_=ctx_sb
            )

        pend = None
        for b in range(BPC):
            enc_sb = enc_pool.tile([P, KT, S], f32r)
            if b == 0:
                # chunked + interleaved with W2T so the first energy matmuls
                # start as soon as the first chunk pair lands
                for jk in range(KT):
                    nc.sync.dma_start(
                        out=enc_sb[:, jk, :], in_=encT[b, ts(jk, P), :].bitcast(f32r)
                    )
                    nc.sync.dma_start(
                        out=w2t_sb[:, jk, :], in_=w2t[ts(jk, P), :].bitcast(f32r)
                    )
            else:
                nc.sync.dma_start(
                    out=enc_sb,
                    in_=encT[b].rearrange("(j p) s -> p j s", p=P).bitcast(f32r),
                )
            et_sb = et_pool.tile([P, HT, S], f32r)
            sc_ps = scps_pool.tile([1, S], f32)
            for jh in range(HT):
                ps = eps_pool.tile([P, S], f32)
                for jk in range(KT):
                    nc.tensor.matmul(
                        ps,
                        lhsT=w2t_sb[:, jk, ts(jh, P)],
                        rhs=enc_sb[:, jk, :],
                        start=(jk == 0),
                        stop=(jk == KT - 1),
                    )
                nc.scalar.activation(
                    out=et_sb[:, jh, :],
                    in_=ps,
                    func=Act.Tanh,
                    bias=hqbT[:, jh, b : b + 1],
                    scale=1.0,
                )
            for jh in range(HT):
                nc.tensor.matmul(
                    sc_ps,
                    lhsT=sm_sb[:, jh, BPC + 1 : BPC + 2],
                    rhs=et_sb[:, jh, :],
                    start=(jh == 0),
                    stop=(jh == HT - 1),
                )
            # epilogue of the previous batch lands here so its PE work (the
            # broadcast matmul) never stalls on the softmax round-trip
            if pend is not None:
                emit_epilogue(*pend)
            pend = (b, enc_sb, sc_ps)
        emit_epilogue(*pend)

    return nc


def _get_nc():
    if "nc" not in _COMPILED:
        _install_tile_patch()
        _install_profile_shim()
        nc = _build_nc()
        _split_excess_waits(nc)
        _COMPILED["nc"] = nc
    return _COMPILED["nc"]


def kernel(hidden, encoder_outputs, W, b, v):
    global LAST_EXEC_TIME_NS, LAST_RESULTS
    from concourse.bass_utils import run_bass_kernel_spmd

    hidden = np.ascontiguousarray(np.asarray(hidden, dtype=np.float32))
    enc = np.ascontiguousarray(np.asarray(encoder_outputs, dtype=np.float32))
    W = np.ascontiguousarray(np.asarray(W, dtype=np.float32))
    b = np.ascontiguousarray(np.asarray(b, dtype=np.float32))
    v = np.ascontiguousarray(np.asarray(v, dtype=np.float32))

    w1t = np.ascontiguousarray(W[:, :H].T)  # [k, h]
    w2t = np.ascontiguousarray(W[:, H:].T)  # [k, h]

    in_maps = []
    for c in range(N_CORES):
        lo, hi = c * BPC, (c + 1) * BPC
        in_maps.append(
            {
                "encT": np.ascontiguousarray(enc[lo:hi].transpose(0, 2, 1)),
                "w2t": w2t,
                "w1t": w1t,
                "smalls": np.ascontiguousarray(
                    np.concatenate(
                        [hidden[lo:hi].T, b.reshape(H, 1), v.reshape(H, 1)], axis=1
                    )
                ),
                "onesrow": np.ones((1, P), dtype=np.float32),
            }
        )

    nc = _get_nc()
    trace = bool(os.environ.get("KERNEL_TRACE"))
    res = run_bass_kernel_spmd(
        nc,
        in_maps,
        list(range(N_CORES)),
        trace=trace,
        tmpdir=os.environ.get("KERNEL_TRACE_DIR") or None,
    )
    LAST_EXEC_TIME_NS = res.exec_time_ns
    LAST_RESULTS = res

    context = np.empty((B, 1, H), dtype=np.float32)
    attn = np.empty((B, 1, S), dtype=np.float32)
    for c in range(N_CORES):
        o = res.results[c]["out"]  # [BPC, H+S]
        lo = c * BPC
        # ctx section is stored [p, j] p-major with h = j*128 + p
        context[lo : lo + BPC, 0, :] = (
            o[:, :H].reshape(BPC, P, HT).transpose(0, 2, 1).reshape(BPC, H)
        )
        attn[lo : lo + BPC, 0, :] = o[:, H:]
    return context, attn


# revision 14
# speedup vs baseline: 1.1686x; 1.0101x over previous
"""Bahdanau attention layer on 8 Trainium2 NeuronCores, data-parallel over batch.

Reference computation (B=64, S=512, H=1024):
    cat    = concat([hidden_bcast, encoder_outputs], -1)        # [B,S,2H]
    energy = tanh(cat @ W.T + b)                                # [B,S,H]
    scores = energy @ v                                         # [B,S]
    attn   = softmax(scores, axis=-1)                           # [B,1,S]
    ctx    = attn @ encoder_outputs                             # [B,1,H]

Sharding: batch is split 8 ways (8 batches per core); W/b/v replicated.
Per core the dominant work is the energy matmul, computed as
P.T[h,s] = sum_k W2T[k,h] * encT[k,s] with fp32r (FP22) matmuls so the
PE streams one column per cycle.  hidden@W1.T+b collapses to a per-(h,b)
bias folded into the tanh activation.  scores are v.T @ tanh-tiles on the
PE; softmax runs on DVE/ACT; context is a fused multiply-reduce on DVE
against a PE-broadcast of the attention row.
"""

import os
import sys

if "/opt/trn_rl_repo" not in sys.path:
    sys.path.insert(0, "/opt/trn_rl_repo")

import numpy as np

B, S, H = 64, 512, 1024
N_CORES = 8
BPC = B // N_CORES
P = 128
KT = H // P  # k tiles (contraction)
HT = H // P  # h tiles (output hidden)

LAST_EXEC_TIME_NS = None
LAST_RESULTS = None

_COMPILED = {}


def _install_tile_patch():
    """This image's walrus rejects instructions with 3+ semaphore waits; Tile's
    exit drain collects one wait per active proc.  Split them across a chain of
    single-wait drains."""
    import concourse.tile as tile
    from concourse.vector_clock import ScopedClock

    if getattr(tile.TileContext, "_drain_patch_installed", False):
        return

    def _patched_drain_and_barrier(self, tick_clock, wait_clock):
        nc = self.nc
        vc = tick_clock.global_clock
        for proc in range(len(vc)):
            tick = vc[proc]
            if tick <= 0:
                continue
            d = nc.sync.drain()
            sc = ScopedClock()
            sc.require_at_least(None, proc, tick)
            wait_clock.add_sem_waits(d.ins, sc)
        nc.sync.drain()
        nc.all_engine_barrier()
        assert self.sems is not None
        popped = nc._tile_sem_poison_stack.pop()
        assert popped is self._sem_poison
        nc.clear_and_free_semaphores(list(self.sems.allocated().values()))
        nc.all_engine_barrier()

    tile.TileContext._drain_and_barrier = _patched_drain_and_barrier
    tile.TileContext._drain_patch_installed = True


def _split_excess_waits(nc, limit=1):
    """This image's walrus rejects instructions carrying more than ~2 semaphore
    waits ("Too many sync wait commands").  Move excess waits onto InstNoOp
    carriers inserted immediately before the instruction on the same engine —
    per-engine program order makes the carrier's waits complete first."""
    from concourse import mybir

    n_carriers = 0
    for f in nc.m.functions:
        for bb in f.blocks:
            insts = bb.instructions
            idx = 0
            while idx < len(insts):
                inst = insts[idx]
                si = inst.sync_info
                if si is None or len(si.on_wait) <= limit:
                    idx += 1
                    continue
                waits = list(si.on_wait)
                si.on_wait = waits[-limit:]
                extra = waits[:-limit]
                pos = idx
                for lo in range(0, len(extra), limit):
                    n_carriers += 1
                    nop = mybir.InstNoOp(
                        name=f"I-waitcarrier-{n_carriers}",
                        engine=inst.engine,
                        ins=[],
                        outs=[],
                    )
                    nop.sync_info = mybir.SyncInfo(
                        on_wait=extra[lo : lo + limit], on_update=[]
                    )
                    insts.insert(pos, nop)
                    pos += 1
                    idx += 1
                idx += 1
    return n_carriers


def _install_profile_shim():
    """antenv.axon_hooks is absent from this image; recreate it and register the
    ctypes NTFF hook so run_bass_kernel_spmd(trace=True) can profile."""
    import types

    if "antenv.axon_hooks" in sys.modules:
        return
    mod = types.ModuleType("antenv.axon_hooks")
    mod._hook = None
    mod.set_axon_ntff_profile_hook = lambda h: setattr(mod, "_hook", h)
    mod.get_axon_ntff_profile_hook = lambda: mod._hook
    sys.modules["antenv.axon_hooks"] = mod
    try:
        from trn_agent_boot.trn_boot import _ntff_profile_via_ctypes

        mod._hook = _ntff_profile_via_ctypes("/opt/axon/libaxon_pjrt.so")
    except Exception:
        pass


def _build_nc():
    import concourse.bass as bass
    import concourse.tile as tile
    from concourse import mybir
    from concourse.bass import ts
    from contextlib import ExitStack

    f32 = mybir.dt.float32
    f32r = mybir.dt.float32r
    Alu = mybir.AluOpType
    Act = mybir.ActivationFunctionType
    Ax = mybir.AxisListType

    nc = bass.Bass()
    encT = nc.declare_dram_parameter("encT", [BPC, H, S], f32, isOutput=False)
    w2t = nc.declare_dram_parameter("w2t", [H, H], f32, isOutput=False)
    w1t = nc.declare_dram_parameter("w1t", [H, H], f32, isOutput=False)
    smalls = nc.declare_dram_parameter("smalls", [H, BPC + 2], f32, isOutput=False)
    onesrow = nc.declare_dram_parameter("onesrow", [1, P], f32, isOutput=False)
    out = nc.declare_dram_parameter("out", [BPC, H + S], f32, isOutput=True)

    with tile.TileContext(nc) as tc, ExitStack() as ctx:
        singles = ctx.enter_context(tc.tile_pool(name="singles", bufs=1))
        # One DMA for all the small tensors (hT cols 0..7, b col 8, v col 9):
        # tiny transfers each cost ~600ns of SP issue, so batch them.
        sm_sb = singles.tile([P, KT, BPC + 2], f32r)
        nc.sync.dma_start(
            out=sm_sb,
            in_=smalls.rearrange("(j p) c -> p j c", p=P).bitcast(f32r),
        )
        ones_sb = singles.tile([1, P], f32r)
        nc.sync.dma_start(out=ones_sb, in_=onesrow[:, :].bitcast(f32r))
        # per-(h, b) tanh bias: hqbT[p, jh, b] = (hidden @ W1.T + b)[b, jh*128+p]
        hqbT = singles.tile([P, HT, BPC], f32)
        w2t_sb = singles.tile([P, KT, H], f32r)

        with tc.tile_pool(name="w1pool", bufs=1) as w1pool, tc.tile_pool(
            name="hqps", bufs=2, space="PSUM"
        ) as hqps:
            w1_sb = w1pool.tile([P, KT, H], f32r)
            for jk in range(KT):
                nc.sync.dma_start(
                    out=w1_sb[:, jk, :], in_=w1t[ts(jk, P), :].bitcast(f32r)
                )
            for jh in range(HT):
                ps = hqps.tile([P, BPC], f32)
                for jk in range(KT):
                    nc.tensor.matmul(
                        ps,
                        lhsT=w1_sb[:, jk, ts(jh, P)],
                        rhs=sm_sb[:, jk, 0:BPC],
                        start=(jk == 0),
                        stop=(jk == KT - 1),
                    )
                nc.vector.tensor_scalar(
                    out=hqbT[:, jh, :],
                    in0=ps,
                    scalar1=sm_sb[:, jh, BPC : BPC + 1].bitcast(f32),
                    scalar2=None,
                    op0=Alu.add,
                )

        enc_pool = ctx.enter_context(tc.tile_pool(name="enc", bufs=4))
        et_pool = ctx.enter_context(tc.tile_pool(name="et", bufs=2))
        eps_pool = ctx.enter_context(tc.tile_pool(name="eps", bufs=2, space="PSUM"))
        scps_pool = ctx.enter_context(tc.tile_pool(name="scps", bufs=2, space="PSUM"))
        bcps_pool = ctx.enter_context(tc.tile_pool(name="bcps", bufs=2, space="PSUM"))
        sm_pool = ctx.enter_context(tc.tile_pool(name="sm", bufs=2))
        scr_pool = ctx.enter_context(tc.tile_pool(name="scr", bufs=2))
        ctxo_pool = ctx.enter_context(tc.tile_pool(name="ctxo", bufs=2))

        def emit_epilogue(b, enc_sb, sc_ps):
            # softmax over the 512 scores (single partition)
            nm = sm_pool.tile([1, 1], f32, tag="nm")
            nc.vector.reduce_max(out=nm, in_=sc_ps, axis=Ax.X, negate=True)
            a_sb = sm_pool.tile([1, S], f32r, tag="a")
            ssum = sm_pool.tile([1, 1], f32, tag="ssum")
            nc.scalar.activation(
                out=a_sb, in_=sc_ps, func=Act.Exp, bias=nm, scale=1.0, accum_out=ssum
            )
            rs = sm_pool.tile([1, 1], f32, tag="rs")
            nc.vector.reciprocal(out=rs, in_=ssum)
            nc.vector.tensor_scalar_mul(out=a_sb, in0=a_sb, scalar1=rs)
            nc.gpsimd.dma_start(out=out[b : b + 1, H : H + S], in_=a_sb.bitcast(f32))
            # broadcast attention row to 128 partitions via K=1 matmul
            bc_ps = bcps_pool.tile([P, S], f32)
            nc.tensor.matmul(
                bc_ps,
                lhsT=ones_sb,
                rhs=a_sb,
                start=True,
                stop=True,
            )
            # context: ctxT[c] = sum_s encT[c, s] * attn[s], fused mul+reduce
            ctx_sb = ctxo_pool.tile([P, HT], f32)
            for jh in range(HT):
                scr = scr_pool.tile([P, S], f32)
                nc.vector.tensor_mul(
                    out=scr, in0=enc_sb[:, jh, :].bitcast(f32), in1=bc_ps
                )
                nc.vector.reduce_sum(
                    out=ctx_sb[:, jh : jh + 1], in_=scr, axis=Ax.X
                )
            # ctx written p-major: out[b, p*HT + j] = ctxT[p, j]; host unpermutes
            nc.gpsimd.dma_start(
                out=out[b, 0:H].rearrange("(p j) -> p j", j=HT), in_=ctx_sb
            )

        def emit_scores(b, et_sb):
            sc_ps = scps_pool.tile([1, S], f32)
            for jh in range(HT):
                nc.tensor.matmul(
                    sc_ps,
                    lhsT=sm_sb[:, jh, BPC + 1 : BPC + 2],
                    rhs=et_sb[:, jh, :],
                    start=(jh == 0),
                    stop=(jh == HT - 1),
                )
            return sc_ps

        # 2-deep pipeline: iteration b emits energy(b), scores(b-1),
        # epilogue(b-2) so the PE never waits on tanh or softmax round-trips.
        sc_wait = None   # (b, enc_sb, et_sb) awaiting scores
        epi_wait = None  # (b, enc_sb, sc_ps) awaiting epilogue
        for b in range(BPC):
            enc_sb = enc_pool.tile([P, KT, S], f32r)
            if b == 0:
                # chunked + interleaved with W2T so the first energy matmuls
                # start as soon as the first chunk pair lands
                for jk in range(KT):
                    nc.sync.dma_start(
                        out=enc_sb[:, jk, :], in_=encT[b, ts(jk, P), :].bitcast(f32r)
                    )
                    nc.sync.dma_start(
                        out=w2t_sb[:, jk, :], in_=w2t[ts(jk, P), :].bitcast(f32r)
                    )
            else:
                nc.sync.dma_start(
                    out=enc_sb,
                    in_=encT[b].rearrange("(j p) s -> p j s", p=P).bitcast(f32r),
                )
            et_sb = et_pool.tile([P, HT, S], f32r)
            for jh in range(HT):
                ps = eps_pool.tile([P, S], f32)
                for jk in range(KT):
                    nc.tensor.matmul(
                        ps,
                        lhsT=w2t_sb[:, jk, ts(jh, P)],
                        rhs=enc_sb[:, jk, :],
                        start=(jk == 0),
                        stop=(jk == KT - 1),
                    )
                nc.scalar.activation(
                    out=et_sb[:, jh, :],
                    in_=ps,
                    func=Act.Tanh,
                    bias=hqbT[:, jh, b : b + 1],
                    scale=1.0,
                )
            if sc_wait is not None:
                sb, senc, set_ = sc_wait
                sc_ps = emit_scores(sb, set_)
                new_epi = (sb, senc, sc_ps)
            else:
                new_epi = None
            if epi_wait is not None:
                emit_epilogue(*epi_wait)
            epi_wait = new_epi
            sc_wait = (b, enc_sb, et_sb)
        sb, senc, set_ = sc_wait
        sc_ps = emit_scores(sb, set_)
        if epi_wait is not None:
            emit_epilogue(*epi_wait)
        emit_epilogue(sb, senc, sc_ps)

    return nc


def _get_nc():
    if "nc" not in _COMPILED:
        _install_tile_patch()
        _install_profile_shim()
        nc = _build_nc()
        _split_excess_waits(nc)
        _COMPILED["nc"] = nc
    return _COMPILED["nc"]


def kernel(hidden, encoder_outputs, W, b, v):
    global LAST_EXEC_TIME_NS, LAST_RESULTS
    from concourse.bass_utils import run_bass_kernel_spmd

    hidden = np.ascontiguousarray(np.asarray(hidden, dtype=np.float32))
    enc = np.ascontiguousarray(np.asarray(encoder_outputs, dtype=np.float32))
    W = np.ascontiguousarray(np.asarray(W, dtype=np.float32))
    b = np.ascontiguousarray(np.asarray(b, dtype=np.float32))
    v = np.ascontiguousarray(np.asarray(v, dtype=np.float32))

    w1t = np.ascontiguousarray(W[:, :H].T)  # [k, h]
    w2t = np.ascontiguousarray(W[:, H:].T)  # [k, h]

    in_maps = []
    for c in range(N_CORES):
        lo, hi = c * BPC, (c + 1) * BPC
        in_maps.append(
            {
                "encT": np.ascontiguousarray(enc[lo:hi].transpose(0, 2, 1)),
                "w2t": w2t,
                "w1t": w1t,
                "smalls": np.ascontiguousarray(
                    np.concatenate(
                        [hidden[lo:hi].T, b.reshape(H, 1), v.reshape(H, 1)], axis=1
                    )
                ),
                "onesrow": np.ones((1, P), dtype=np.float32),
            }
        )

    nc = _get_nc()
    trace = bool(os.environ.get("KERNEL_TRACE"))
    res = run_bass_kernel_spmd(
        nc,
        in_maps,
        list(range(N_CORES)),
        trace=trace,
        tmpdir=os.environ.get("KERNEL_TRACE_DIR") or None,
    )
    LAST_EXEC_TIME_NS = res.exec_time_ns
    LAST_RESULTS = res

    context = np.empty((B, 1, H), dtype=np.float32)
    attn = np.empty((B, 1, S), dtype=np.float32)
    for c in range(N_CORES):
        o = res.results[c]["out"]  # [BPC, H+S]
        lo = c * BPC
        # ctx section is stored [p, j] p-major with h = j*128 + p
        context[lo : lo + BPC, 0, :] = (
            o[:, :H].reshape(BPC, P, HT).transpose(0, 2, 1).reshape(BPC, H)
        )
        attn[lo : lo + BPC, 0, :] = o[:, H:]
    return context, attn
